# revision 2
# baseline (speedup 1.0000x reference)
"""Trainium2 Bass kernel for the annealed mean-field Boltzmann machine.

Strategy: 1D tensor-parallel over 8 NeuronCores. Each core holds a
256-column shard of hh/vis_hid and a 512-column shard of vv/vis_hid.T,
all SBUF-resident in fp32 (the dynamics are chaotic: any reduced-
precision matmul input — bf16, fp16, fp8, or the fp32r fast path at
~1.5e-4 — amplifies to O(1) final error, measured both in numpy
emulation and with an on-device fp32r probe; fp32's ~1.9e-7 is
required, so matmuls run at the fp32 4-cycle/row rate).

Schedule: the per-step critical ring is mix_h -> AllGather(hid) ->
vht -> mix_v -> AllGather(vis half A/B) -> vh(next step). Biases are
folded into the scalar-engine sigmoid as per-partition bias/temp
tables (removes 6 rank-1 matmuls/step); each AllGather chain uses one
SBUF->DRAM bounce + collective + chunked restage, with vv matmul
blocks placed to cover each chain's latency (F/L/W2 split and restage
ladders tuned against the cost-model timeline).

States are kept transposed (feature-on-partition, batch-on-free).
Every field matmul uses the weight tile as the stationary operand
(128x128) and a state k-tile (128x64) as the moving operand:
out[feat_tile, batch] += W[k, feat_tile].T @ stateT[k]. Outputs come
out feature-major, exactly the layout the next step needs, so there are
no transposes anywhere. Bias enters as a rank-1 matmul (bias x ones),
sigmoid/(1/temp) on the scalar engine, 0.9/0.1 mixing on the vector
engine, and each core's state shard is AllGathered so every core has
the full state for the next half-step.
"""

import sys
import time

sys.path.insert(0, "/opt/trn_rl_repo")

import numpy as np

N_CORES = 8
V_SIZE = 4096
H_SIZE = 2048
BATCH = 64
HS = H_SIZE // N_CORES  # 256 hid cols per core
VS = V_SIZE // N_CORES  # 512 vis cols per core
KT_H = H_SIZE // 128  # 16 k-tiles over hid features
KT_V = V_SIZE // 128  # 32 k-tiles over vis features
NTH = HS // 128  # 2 feature out-tiles per core (hid)
NTV = VS // 128  # 4 feature out-tiles per core (vis)

_BUILT = {}


def _build(n_steps: int, temps: np.ndarray, sim_mode: bool = False,
           no_comm: bool = False, splits=((7, 5, 0), (5, 6, 5), (5, 6, 5))):
    import concourse.bacc as bacc
    import concourse.tile as tile
    import concourse.mybir as mybir

    F32 = mybir.dt.float32
    SIG = mybir.ActivationFunctionType.Sigmoid
    MULT = mybir.AluOpType.mult
    ADD = mybir.AluOpType.add

    nc = bacc.Bacc(
        "TRN2",
        target_bir_lowering=False,
        debug=False,
        enable_asserts=True,
        num_devices=1 if sim_mode else N_CORES,
    )

    def din(name, shape):
        return nc.dram_tensor(name, shape, F32, kind="ExternalInput").ap()

    xT = din("xT", [V_SIZE, BATCH])
    xT_my = din("xT_my", [VS, BATCH])
    hid0T = din("hid0T", [H_SIZE, BATCH])
    hh_w = din("hh_w", [H_SIZE, HS])
    vh_w = din("vh_w", [V_SIZE, HS])
    vv_w = din("vv_w", [V_SIZE, VS])
    vht_w = din("vht_w", [H_SIZE, VS])
    # bias_over_temp tables: [128, NT * n_steps], value bias[p,j]/temps[i]
    hbt_in = din("hbt_in", [128, NTH * n_steps])
    vbt_in = din("vbt_in", [128, NTV * n_steps])
    out_vis = nc.dram_tensor(
        "vis_shT", [VS, BATCH], F32, kind="ExternalOutput"
    ).ap()

    rg = [list(range(N_CORES))]
    shared_as = "Local" if sim_mode else "Shared"

    def all_gather(src_sbuf, ag_out, scratch, eng=None):
        """Gather the SBUF shard `src_sbuf` ([128, nt, B], feature-major)
        into the full-state DRAM tile `ag_out`.

        Real mode: SBUF -> internal-DRAM bounce, then the AllGather
        collective (runs on TOPSP/SDMA silicon, off all five engines).

        Sim mode: a 2-DMA chain through a scratch DRAM tile standing in
        for bounce + collective. Modeled latency (~2.8us + ~3.0us + the
        downstream restage ~2.5us) is conservative vs the measured real
        chain (~1us bounce + 4.6-5.2us 8-core AllGather + restage). Each
        chain gets its own otherwise-idle engine queue, mirroring the
        real concurrency of the collective hardware."""
        rows = 128 * src_sbuf.shape[1]
        all_gather.n = getattr(all_gather, "n", 0) + 1
        if not sim_mode:
            ag_in = dram.tile(
                [rows, BATCH], F32, name=f"agi{all_gather.n}", tag="agi"
            )
            nc.scalar.dma_start(
                ag_in[:].rearrange("(j p) n -> p j n", p=128), src_sbuf[:]
            )
            nc.gpsimd.collective_compute(
                "AllGather",
                mybir.AluOpType.bypass,
                replica_groups=rg,
                ins=[ag_in[:].opt()],
                outs=[ag_out[:].opt()],
            )
        else:
            eng = eng or nc.gpsimd
            eng.dma_start(
                scratch[0:rows, :].rearrange("(j p) n -> p j n", p=128),
                src_sbuf[:],
            )
            eng.dma_start(ag_out[:], scratch[:])

    with tile.TileContext(nc) as tc:
        with (
            tc.tile_pool(name="w", bufs=1) as wpool,
            tc.tile_pool(name="st", bufs=1) as stpool,
            tc.tile_pool(name="act", bufs=3) as actpool,
            tc.tile_pool(name="ps_h", bufs=2, space="PSUM") as ps_h,
            tc.tile_pool(name="ps_v", bufs=4, space="PSUM") as ps_v,
            tc.tile_pool(name="dram", bufs=2, space="DRAM") as dram,
        ):
            # --- weights (SBUF-resident), blocked [k, j] 128x128 ---
            hh_sb = wpool.tile([128, KT_H, NTH, 128], F32)
            vh_sb = wpool.tile([128, KT_V, NTH, 128], F32)
            vv_sb = wpool.tile([128, KT_V, NTV, 128], F32)
            vht_sb = wpool.tile([128, KT_H, NTV, 128], F32)
            for j in range(0, KT_H, 4):
                nc.sync.dma_start(
                    hh_sb[:, j : j + 4, :, :],
                    hh_w.rearrange("(k p) (j n) -> p k j n", p=128, n=128)[
                        :, j : j + 4, :, :
                    ],
                )
                nc.sync.dma_start(
                    vht_sb[:, j : j + 4, :, :],
                    vht_w.rearrange("(k p) (j n) -> p k j n", p=128, n=128)[
                        :, j : j + 4, :, :
                    ],
                )
            for j in range(0, KT_V, 4):
                nc.sync.dma_start(
                    vh_sb[:, j : j + 4, :, :],
                    vh_w.rearrange("(k p) (j n) -> p k j n", p=128, n=128)[
                        :, j : j + 4, :, :
                    ],
                )
                nc.sync.dma_start(
                    vv_sb[:, j : j + 4, :, :],
                    vv_w.rearrange("(k p) (j n) -> p k j n", p=128, n=128)[
                        :, j : j + 4, :, :
                    ],
                )

            # --- bias/temp tables (fold bias into the activation) ---
            hbt_sb = wpool.tile([128, NTH, n_steps], F32)
            vbt_sb = wpool.tile([128, NTV, n_steps], F32)
            nc.sync.dma_start(
                hbt_sb[:], hbt_in.rearrange("p (j i) -> p j i", j=NTH)
            )
            nc.sync.dma_start(
                vbt_sb[:], vbt_in.rearrange("p (j i) -> p j i", j=NTV)
            )

            # --- states (transposed: feature-on-partition) ---
            visT = stpool.tile([128, KT_V, BATCH], F32)
            hidT = stpool.tile([128, KT_H, BATCH], F32)
            vmyT = stpool.tile([128, NTV, BATCH], F32)
            hmyT = stpool.tile([128, NTH, BATCH], F32)
            for j in range(0, KT_V, 8):
                nc.sync.dma_start(
                    visT[:, j : j + 8, :],
                    xT.rearrange("(k p) n -> p k n", p=128)[:, j : j + 8, :],
                )
            nc.sync.dma_start(vmyT[:], xT_my.rearrange("(k p) n -> p k n", p=128))
            nc.sync.dma_start(hidT[:], hid0T.rearrange("(k p) n -> p k n", p=128))
            nc.sync.dma_start(
                hmyT[:],
                hid0T.rearrange("(k p) n -> p k n", p=128)[:, :NTH, :],
            )

            # vis and hid k-tiles are stored in AllGather output order
            # (host-side permutation), so restage is a contiguous copy and
            # consuming k in ascending order reads the early half first
            scr_h0 = dram.tile([H_SIZE, BATCH], F32, name="scr_h0", bufs=1)
            scr_h1 = dram.tile([H_SIZE // 2, BATCH], F32, name="scr_h1", bufs=1)
            scr_v0 = dram.tile([H_SIZE, BATCH], F32, name="scr_v0", bufs=1)
            scr_v1 = dram.tile([H_SIZE, BATCH], F32, name="scr_v1", bufs=1)
            scr_h = [scr_h0, scr_h1]
            scr_v = [scr_v0, scr_v1]
            KH2 = KT_H // 2  # 8: k-tiles per hid AllGather half
            KV2 = KT_V // 2  # 16: k-tiles per vis AllGather half
            # restage chunk ladder: small first chunk lands fast so the
            # first consumer matmuls start early
            RESTAGE_H = splits[1] if splits[1] else (2, 14)
            RESTAGE_V = splits[2] if splits[2] else (2, 14)

            for i in range(n_steps):
                inv_t = float(1.0 / temps[i])
                last = i == n_steps - 1

                # ---- field matmuls, ordered so that every comm chain is
                # covered by matmuls that do not depend on it ----
                phs = [
                    ps_h.tile([128, BATCH], F32, name=f"ph{i}_{j}", tag="ph")
                    for j in range(NTH)
                ]
                pvs = [
                    ps_v.tile([128, BATCH], F32, name=f"pv{i}_{j}", tag="pv")
                    for j in range(NTV)
                ]
                # schedule params: F = vv-j23 A-half k-tiles filling the
                # wait for the vis-B restage; L = vv-j23 B-half k-tiles
                # squeezed before vht23 (cover for AG_v_A); IH = interleave
                # hh into vh-A so consumption tracks the restage rung pace
                F, L = splits[0][0], splits[0][1]
                IH = len(splits[0]) > 2 and splits[0][2]
                if IH:
                    for k in range(KT_H):
                        for j in range(NTH):
                            nc.tensor.matmul(
                                phs[j][:], hh_sb[:, k, j, :], hidT[:, k, :],
                                start=(k == 0), stop=False,
                            )
                        for j in range(NTH):
                            nc.tensor.matmul(
                                phs[j][:], vh_sb[:, k, j, :], visT[:, k, :],
                                start=False, stop=False,
                            )
                else:
                    for k in range(KT_H):
                        for j in range(NTH):
                            nc.tensor.matmul(
                                phs[j][:], hh_sb[:, k, j, :], hidT[:, k, :],
                                start=(k == 0), stop=False,
                            )
                # vh A-half (unblocked by AG_v_A(i-1) restage)
                for k in range(KT_H if IH else 0, KV2):
                    for j in range(NTH):
                        nc.tensor.matmul(
                            phs[j][:], vh_sb[:, k, j, :], visT[:, k, :],
                            start=False, stop=False,
                        )
                # FILL: vv j2/j3 on A-half k-tiles while the B restage lands
                for k in range(F):
                    for j in (2, 3):
                        nc.tensor.matmul(
                            pvs[j][:], vv_sb[:, k, j, :], visT[:, k, :],
                            start=(k == 0), stop=False,
                        )
                # vh B-half (needs AG_v_B(i-1) restage)
                for k in range(KV2, KT_V):
                    for j in range(NTH):
                        nc.tensor.matmul(
                            phs[j][:], vh_sb[:, k, j, :], visT[:, k, :],
                            start=False, stop=(k == KT_V - 1),
                        )
                # hid field complete: sigmoid + mix + AllGather
                for j in range(NTH):
                    ph = phs[j]
                    prob = actpool.tile(
                        [128, BATCH], F32, name=f"prh{i}_{j}", tag="pr"
                    )
                    nc.scalar.activation(
                        prob[:], ph[:], SIG,
                        bias=hbt_sb[:, j, i : i + 1], scale=inv_t,
                    )
                    tmp = actpool.tile(
                        [128, BATCH], F32, name=f"tmh{i}_{j}", tag="tm"
                    )
                    nc.vector.tensor_sub(tmp[:], prob[:], hmyT[:, j, :])
                    nc.vector.scalar_tensor_tensor(
                        hmyT[:, j, :], tmp[:], 0.1, hmyT[:, j, :], MULT, ADD
                    )
                if not no_comm:
                    ag_out_h = dram.tile(
                        [H_SIZE, BATCH], F32, addr_space=shared_as,
                        name=f"agoh{i}", tag="agoh",
                    )
                    all_gather(hmyT[:, :, :], ag_out_h, scr_h[0], nc.gpsimd)
                    qs = 0
                    for w in RESTAGE_H:
                        nc.sync.dma_start(
                            hidT[:, qs : qs + w, :],
                            ag_out_h[:].rearrange("(k p) n -> p k n", p=128)[
                                :, qs : qs + w, :
                            ],
                        )
                        qs += w
                # W2 cover for the hid AllGather chain: all vv j0/j1 plus
                # the middle vv j2/j3 k-tiles
                for k in range(KT_V):
                    for j in (0, 1):
                        nc.tensor.matmul(
                            pvs[j][:], vv_sb[:, k, j, :], visT[:, k, :],
                            start=(k == 0), stop=False,
                        )
                for k in range(F, KT_V - L):
                    for j in (2, 3):
                        nc.tensor.matmul(
                            pvs[j][:], vv_sb[:, k, j, :], visT[:, k, :],
                            start=(F == 0 and k == 0), stop=False,
                        )
                # hid(i)-dependent part of the vis field, then mix + AG per
                # j-pair; the vv j2/j3 tail (L tiles) runs between the two
                # halves as cover for AG_v_A
                for half in range(2):
                    js = (0, 1) if half == 0 else (2, 3)
                    for k in range(KT_H):
                        for j in js:
                            nc.tensor.matmul(
                                pvs[j][:], vht_sb[:, k, j, :], hidT[:, k, :],
                                start=False, stop=(k == KT_H - 1),
                            )
                    if half == 0:
                        for k in range(KT_V - L, KT_V):
                            for j in (2, 3):
                                nc.tensor.matmul(
                                    pvs[j][:], vv_sb[:, k, j, :], visT[:, k, :],
                                    start=False, stop=False,
                                )
                    for j in js:
                        pv = pvs[j]
                        prob = actpool.tile(
                            [128, BATCH], F32, name=f"prv{i}_{j}", tag="pr"
                        )
                        nc.scalar.activation(
                            prob[:], pv[:], SIG,
                            bias=vbt_sb[:, j, i : i + 1], scale=inv_t,
                        )
                        tmp = actpool.tile(
                            [128, BATCH], F32, name=f"tmv{i}_{j}", tag="tm"
                        )
                        nc.vector.tensor_sub(tmp[:], prob[:], vmyT[:, j, :])
                        nc.vector.scalar_tensor_tensor(
                            vmyT[:, j, :], tmp[:], 0.1, vmyT[:, j, :], MULT, ADD
                        )
                    if last or no_comm:
                        continue
                    ag_out = dram.tile(
                        [H_SIZE, BATCH], F32, addr_space=shared_as,
                        name=f"agov{i}_{half}", tag="agov",
                    )
                    all_gather(
                        vmyT[:, 2 * half : 2 * half + 2, :], ag_out,
                        scr_v[half],
                        nc.sync if half == 0 else nc.scalar,
                    )
                    qs = 0
                    for w in RESTAGE_V:
                        nc.sync.dma_start(
                            visT[:, KV2 * half + qs : KV2 * half + qs + w, :],
                            ag_out[:].rearrange("(k p) n -> p k n", p=128)[
                                :, qs : qs + w, :
                            ],
                        )
                        qs += w

            nc.sync.dma_start(
                out_vis[:].rearrange("(k p) n -> p k n", p=128), vmyT[:]
            )

    nc.compile()
    return nc


# vis k-tile permutation: SBUF order k' = AllGather output order.
# k' in [0,16): half A = each core's feature tiles {0,1};  orig k = 4c+t
# k' in [16,32): half B = tiles {2,3};                      orig k = 4c+2+t
_PERM_V = [4 * (k % 16 // 2) + (2 * (k // 16)) + (k % 2) for k in range(32)]
# hid k-tile permutation: per-j AllGather j=0 gathers each core's tile 0
# (orig 2c) into k' = c, j=1 gathers tile 1 (orig 2c+1) into k' = 8+c
_PERM_H = [2 * k for k in range(8)] + [2 * k + 1 for k in range(8)]


def _permute_vis_rows(a):
    """Reorder 128-row blocks of a (4096, ...) array into gather order."""
    blocks = a.reshape(32, 128, *a.shape[1:])
    return np.ascontiguousarray(blocks[_PERM_V].reshape(a.shape))


def _permute_hid_rows(a):
    """Reorder 128-row blocks of a (2048, ...) array into gather order."""
    blocks = a.reshape(16, 128, *a.shape[1:])
    return np.ascontiguousarray(blocks[_PERM_H].reshape(a.shape))


def _prep_inputs(x, vis_bias, hid_bias, vis_hid, vis_vis_raw, hid_hid_raw,
                 temps):
    f32 = np.float32
    n_steps = len(temps)
    vv = np.triu(np.asarray(vis_vis_raw, dtype=f32), 1)
    vv = vv + vv.T
    hh = np.triu(np.asarray(hid_hid_raw, dtype=f32), 1)
    hh = hh + hh.T
    vis_hid = np.ascontiguousarray(np.asarray(vis_hid, dtype=f32))
    vht = np.ascontiguousarray(vis_hid.T)  # (H, V)
    x = np.asarray(x, dtype=f32)
    xT = np.ascontiguousarray(x.T)
    hid0 = np.full((H_SIZE, BATCH), 0.5, dtype=f32)
    hb = np.asarray(hid_bias, dtype=f32)
    vb = np.asarray(vis_bias, dtype=f32)
    inv_t = (1.0 / temps).astype(f32)  # [n_steps]

    in_maps = []
    for c in range(N_CORES):
        hsl = slice(c * HS, (c + 1) * HS)
        vsl = slice(c * VS, (c + 1) * VS)
        # bias_over_temp tables [128, NT, n_steps] -> flat [128, NT*n_steps]
        hbt = (
            hb[hsl].reshape(NTH, 128).T[:, :, None] * inv_t[None, None, :]
        ).astype(f32)
        vbt = (
            vb[vsl].reshape(NTV, 128).T[:, :, None] * inv_t[None, None, :]
        ).astype(f32)
        in_maps.append(
            {
                "xT": _permute_vis_rows(xT),
                "xT_my": np.ascontiguousarray(xT[vsl]),
                "hid0T": hid0,
                "hh_w": np.ascontiguousarray(hh[:, hsl]),
                "vh_w": _permute_vis_rows(np.ascontiguousarray(vis_hid[:, hsl])),
                "vv_w": _permute_vis_rows(np.ascontiguousarray(vv[:, vsl])),
                "vht_w": np.ascontiguousarray(vht[:, vsl]),
                "hbt_in": np.ascontiguousarray(hbt.reshape(128, NTH * n_steps)),
                "vbt_in": np.ascontiguousarray(vbt.reshape(128, NTV * n_steps)),
            }
        )
    return in_maps


def kernel(
    x,
    vis_bias,
    hid_bias,
    vis_hid,
    vis_vis_raw,
    hid_hid_raw,
    max_steps,
):
    from concourse import bass_utils

    n_steps = int(max_steps)
    steps_f = np.float32(n_steps)
    temps = (
        np.float32(0.01)
        * (
            np.float32(1.0)
            + np.float32(4.0)
            * np.exp(
                np.float32(-5.0)
                * np.arange(n_steps, dtype=np.float32)
                / steps_f
            )
        )
    ).astype(np.float32)

    if n_steps not in _BUILT:
        _BUILT[n_steps] = _build(n_steps, temps)
    nc = _BUILT[n_steps]

    in_maps = _prep_inputs(
        x, vis_bias, hid_bias, vis_hid, vis_vis_raw, hid_hid_raw, temps
    )
    res = bass_utils.run_bass_kernel_spmd(
        nc, in_maps, core_ids=list(range(N_CORES))
    )

    out = np.empty((BATCH, V_SIZE), dtype=np.float32)
    for c in range(N_CORES):
        out[:, c * VS : (c + 1) * VS] = res.results[c]["vis_shT"].T
    kernel._last_result = res
    return out



# revision 3
# speedup vs baseline: 1.0035x; 1.0035x over previous
"""Trainium2 Bass kernel for the annealed mean-field Boltzmann machine.

Strategy: 1D tensor-parallel over 8 NeuronCores. Each core holds a
256-column shard of hh/vis_hid and a 512-column shard of vv/vis_hid.T,
all SBUF-resident in fp32 (the dynamics are chaotic: any reduced-
precision matmul input — bf16, fp16, fp8, or the fp32r fast path at
~1.5e-4 — amplifies to O(1) final error, measured both in numpy
emulation and with an on-device fp32r probe; fp32's ~1.9e-7 is
required, so matmuls run at the fp32 4-cycle/row rate).

Schedule: the per-step critical ring is mix_h -> AllGather(hid) ->
vht -> mix_v -> AllGather(vis half A/B) -> vh(next step). Biases are
folded into the scalar-engine sigmoid as per-partition bias/temp
tables (removes 6 rank-1 matmuls/step); each AllGather chain uses one
SBUF->DRAM bounce + collective + chunked restage, with vv matmul
blocks placed to cover each chain's latency (F/L/W2 split and restage
ladders tuned against the cost-model timeline).

States are kept transposed (feature-on-partition, batch-on-free).
Every field matmul uses the weight tile as the stationary operand
(128x128) and a state k-tile (128x64) as the moving operand:
out[feat_tile, batch] += W[k, feat_tile].T @ stateT[k]. Outputs come
out feature-major, exactly the layout the next step needs, so there are
no transposes anywhere. Bias enters as a rank-1 matmul (bias x ones),
sigmoid/(1/temp) on the scalar engine, 0.9/0.1 mixing on the vector
engine, and each core's state shard is AllGathered so every core has
the full state for the next half-step.
"""

import sys
import time

sys.path.insert(0, "/opt/trn_rl_repo")

import numpy as np

N_CORES = 8
V_SIZE = 4096
H_SIZE = 2048
BATCH = 64
HS = H_SIZE // N_CORES  # 256 hid cols per core
VS = V_SIZE // N_CORES  # 512 vis cols per core
KT_H = H_SIZE // 128  # 16 k-tiles over hid features
KT_V = V_SIZE // 128  # 32 k-tiles over vis features
NTH = HS // 128  # 2 feature out-tiles per core (hid)
NTV = VS // 128  # 4 feature out-tiles per core (vis)

_BUILT = {}


def _build(n_steps: int, temps: np.ndarray, sim_mode: bool = False,
           no_comm: bool = False, splits=((5, 5, 0), (5, 6, 5), (5, 6, 5))):
    import concourse.bacc as bacc
    import concourse.tile as tile
    import concourse.mybir as mybir

    F32 = mybir.dt.float32
    SIG = mybir.ActivationFunctionType.Sigmoid
    MULT = mybir.AluOpType.mult
    ADD = mybir.AluOpType.add

    nc = bacc.Bacc(
        "TRN2",
        target_bir_lowering=False,
        debug=False,
        enable_asserts=True,
        num_devices=1 if sim_mode else N_CORES,
    )

    def din(name, shape):
        return nc.dram_tensor(name, shape, F32, kind="ExternalInput").ap()

    xT = din("xT", [V_SIZE, BATCH])
    xT_my = din("xT_my", [VS, BATCH])
    hid0T = din("hid0T", [H_SIZE, BATCH])
    hh_w = din("hh_w", [H_SIZE, HS])
    vh_w = din("vh_w", [V_SIZE, HS])
    vv_w = din("vv_w", [V_SIZE, VS])
    vht_w = din("vht_w", [H_SIZE, VS])
    # bias_over_temp tables: [128, NT * n_steps], value bias[p,j]/temps[i]
    hbt_in = din("hbt_in", [128, NTH * n_steps])
    vbt_in = din("vbt_in", [128, NTV * n_steps])
    out_vis = nc.dram_tensor(
        "vis_shT", [VS, BATCH], F32, kind="ExternalOutput"
    ).ap()

    rg = [list(range(N_CORES))]
    shared_as = "Local" if sim_mode else "Shared"

    def all_gather(src_sbuf, ag_out, scratch, eng=None):
        """Gather the SBUF shard `src_sbuf` ([128, nt, B], feature-major)
        into the full-state DRAM tile `ag_out`.

        Real mode: SBUF -> internal-DRAM bounce, then the AllGather
        collective (runs on TOPSP/SDMA silicon, off all five engines).

        Sim mode: a 2-DMA chain through a scratch DRAM tile standing in
        for bounce + collective. Modeled latency (~2.8us + ~3.0us + the
        downstream restage ~2.5us) is conservative vs the measured real
        chain (~1us bounce + 4.6-5.2us 8-core AllGather + restage). Each
        chain gets its own otherwise-idle engine queue, mirroring the
        real concurrency of the collective hardware."""
        rows = 128 * src_sbuf.shape[1]
        all_gather.n = getattr(all_gather, "n", 0) + 1
        if not sim_mode:
            ag_in = dram.tile(
                [rows, BATCH], F32, name=f"agi{all_gather.n}", tag="agi"
            )
            nc.scalar.dma_start(
                ag_in[:].rearrange("(j p) n -> p j n", p=128), src_sbuf[:]
            )
            nc.gpsimd.collective_compute(
                "AllGather",
                mybir.AluOpType.bypass,
                replica_groups=rg,
                ins=[ag_in[:].opt()],
                outs=[ag_out[:].opt()],
            )
        else:
            eng = eng or nc.gpsimd
            eng.dma_start(
                scratch[0:rows, :].rearrange("(j p) n -> p j n", p=128),
                src_sbuf[:],
            )
            eng.dma_start(ag_out[:], scratch[:])

    with tile.TileContext(nc) as tc:
        with (
            tc.tile_pool(name="w", bufs=1) as wpool,
            tc.tile_pool(name="st", bufs=1) as stpool,
            tc.tile_pool(name="act", bufs=3) as actpool,
            tc.tile_pool(name="ps_h", bufs=2, space="PSUM") as ps_h,
            tc.tile_pool(name="ps_v", bufs=4, space="PSUM") as ps_v,
            tc.tile_pool(name="dram", bufs=2, space="DRAM") as dram,
        ):
            # --- weights (SBUF-resident), blocked [k, j] 128x128 ---
            hh_sb = wpool.tile([128, KT_H, NTH, 128], F32)
            vh_sb = wpool.tile([128, KT_V, NTH, 128], F32)
            vv_sb = wpool.tile([128, KT_V, NTV, 128], F32)
            vht_sb = wpool.tile([128, KT_H, NTV, 128], F32)
            for j in range(0, KT_H, 4):
                nc.sync.dma_start(
                    hh_sb[:, j : j + 4, :, :],
                    hh_w.rearrange("(k p) (j n) -> p k j n", p=128, n=128)[
                        :, j : j + 4, :, :
                    ],
                )
                nc.sync.dma_start(
                    vht_sb[:, j : j + 4, :, :],
                    vht_w.rearrange("(k p) (j n) -> p k j n", p=128, n=128)[
                        :, j : j + 4, :, :
                    ],
                )
            for j in range(0, KT_V, 4):
                nc.sync.dma_start(
                    vh_sb[:, j : j + 4, :, :],
                    vh_w.rearrange("(k p) (j n) -> p k j n", p=128, n=128)[
                        :, j : j + 4, :, :
                    ],
                )
                nc.sync.dma_start(
                    vv_sb[:, j : j + 4, :, :],
                    vv_w.rearrange("(k p) (j n) -> p k j n", p=128, n=128)[
                        :, j : j + 4, :, :
                    ],
                )

            # --- bias/temp tables (fold bias into the activation) ---
            hbt_sb = wpool.tile([128, NTH, n_steps], F32)
            vbt_sb = wpool.tile([128, NTV, n_steps], F32)
            nc.sync.dma_start(
                hbt_sb[:], hbt_in.rearrange("p (j i) -> p j i", j=NTH)
            )
            nc.sync.dma_start(
                vbt_sb[:], vbt_in.rearrange("p (j i) -> p j i", j=NTV)
            )

            # --- states (transposed: feature-on-partition) ---
            # double-buffered: step i reads visTs[i % 2]; the AG_v
            # restage writes visTs[(i + 1) % 2], so it can land without
            # waiting for step i's readers (kills the WAR serialization)
            visTs = [
                stpool.tile([128, KT_V, BATCH], F32, name=f"visT{b}")
                for b in range(2)
            ]
            hidT = stpool.tile([128, KT_H, BATCH], F32)
            vmyT = stpool.tile([128, NTV, BATCH], F32)
            hmyT = stpool.tile([128, NTH, BATCH], F32)
            for j in range(0, KT_V, 8):
                nc.sync.dma_start(
                    visTs[0][:, j : j + 8, :],
                    xT.rearrange("(k p) n -> p k n", p=128)[:, j : j + 8, :],
                )
            nc.sync.dma_start(vmyT[:], xT_my.rearrange("(k p) n -> p k n", p=128))
            nc.sync.dma_start(hidT[:], hid0T.rearrange("(k p) n -> p k n", p=128))
            nc.sync.dma_start(
                hmyT[:],
                hid0T.rearrange("(k p) n -> p k n", p=128)[:, :NTH, :],
            )

            # vis and hid k-tiles are stored in AllGather output order
            # (host-side permutation), so restage is a contiguous copy and
            # consuming k in ascending order reads the early half first
            scr_h0 = dram.tile([H_SIZE, BATCH], F32, name="scr_h0", bufs=1)
            scr_h1 = dram.tile([H_SIZE // 2, BATCH], F32, name="scr_h1", bufs=1)
            scr_v0 = dram.tile([H_SIZE, BATCH], F32, name="scr_v0", bufs=1)
            scr_v1 = dram.tile([H_SIZE, BATCH], F32, name="scr_v1", bufs=1)
            scr_h = [scr_h0, scr_h1]
            scr_v = [scr_v0, scr_v1]
            KH2 = KT_H // 2  # 8: k-tiles per hid AllGather half
            KV2 = KT_V // 2  # 16: k-tiles per vis AllGather half
            # restage chunk ladder: small first chunk lands fast so the
            # first consumer matmuls start early
            RESTAGE_H = splits[1] if splits[1] else (2, 14)
            RESTAGE_V = splits[2] if splits[2] else (2, 14)

            for i in range(n_steps):
                inv_t = float(1.0 / temps[i])
                last = i == n_steps - 1
                visT = visTs[i % 2]
                visW = visTs[(i + 1) % 2]

                # ---- field matmuls, ordered so that every comm chain is
                # covered by matmuls that do not depend on it ----
                phs = [
                    ps_h.tile([128, BATCH], F32, name=f"ph{i}_{j}", tag="ph")
                    for j in range(NTH)
                ]
                pvs = [
                    ps_v.tile([128, BATCH], F32, name=f"pv{i}_{j}", tag="pv")
                    for j in range(NTV)
                ]
                # schedule params: F = vv-j23 A-half k-tiles filling the
                # wait for the vis-B restage; L = vv-j23 B-half k-tiles
                # squeezed before vht23 (cover for AG_v_A); IH = interleave
                # hh into vh-A so consumption tracks the restage rung pace
                F, L = splits[0][0], splits[0][1]
                IH = len(splits[0]) > 2 and splits[0][2]
                if IH:
                    for k in range(KT_H):
                        for j in range(NTH):
                            nc.tensor.matmul(
                                phs[j][:], hh_sb[:, k, j, :], hidT[:, k, :],
                                start=(k == 0), stop=False,
                            )
                        for j in range(NTH):
                            nc.tensor.matmul(
                                phs[j][:], vh_sb[:, k, j, :], visT[:, k, :],
                                start=False, stop=False,
                            )
                else:
                    for k in range(KT_H):
                        for j in range(NTH):
                            nc.tensor.matmul(
                                phs[j][:], hh_sb[:, k, j, :], hidT[:, k, :],
                                start=(k == 0), stop=False,
                            )
                # vh A-half (unblocked by AG_v_A(i-1) restage)
                for k in range(KT_H if IH else 0, KV2):
                    for j in range(NTH):
                        nc.tensor.matmul(
                            phs[j][:], vh_sb[:, k, j, :], visT[:, k, :],
                            start=False, stop=False,
                        )
                # FILL: vv j2/j3 on A-half k-tiles while the B restage lands
                for k in range(F):
                    for j in (2, 3):
                        nc.tensor.matmul(
                            pvs[j][:], vv_sb[:, k, j, :], visT[:, k, :],
                            start=(k == 0), stop=False,
                        )
                # vh B-half (needs AG_v_B(i-1) restage)
                for k in range(KV2, KT_V):
                    for j in range(NTH):
                        nc.tensor.matmul(
                            phs[j][:], vh_sb[:, k, j, :], visT[:, k, :],
                            start=False, stop=(k == KT_V - 1),
                        )
                # hid field complete: sigmoid + mix + AllGather
                for j in range(NTH):
                    ph = phs[j]
                    prob = actpool.tile(
                        [128, BATCH], F32, name=f"prh{i}_{j}", tag="pr"
                    )
                    nc.scalar.activation(
                        prob[:], ph[:], SIG,
                        bias=hbt_sb[:, j, i : i + 1], scale=inv_t,
                    )
                    tmp = actpool.tile(
                        [128, BATCH], F32, name=f"tmh{i}_{j}", tag="tm"
                    )
                    nc.vector.tensor_sub(tmp[:], prob[:], hmyT[:, j, :])
                    nc.vector.scalar_tensor_tensor(
                        hmyT[:, j, :], tmp[:], 0.1, hmyT[:, j, :], MULT, ADD
                    )
                if not no_comm:
                    ag_out_h = dram.tile(
                        [H_SIZE, BATCH], F32, addr_space=shared_as,
                        name=f"agoh{i}", tag="agoh",
                    )
                    all_gather(hmyT[:, :, :], ag_out_h, scr_h[0], nc.gpsimd)
                    qs = 0
                    for w in RESTAGE_H:
                        nc.sync.dma_start(
                            hidT[:, qs : qs + w, :],
                            ag_out_h[:].rearrange("(k p) n -> p k n", p=128)[
                                :, qs : qs + w, :
                            ],
                        )
                        qs += w
                # W2 cover for the hid AllGather chain: all vv j0/j1 plus
                # the middle vv j2/j3 k-tiles
                for k in range(KT_V):
                    for j in (0, 1):
                        nc.tensor.matmul(
                            pvs[j][:], vv_sb[:, k, j, :], visT[:, k, :],
                            start=(k == 0), stop=False,
                        )
                for k in range(F, KT_V - L):
                    for j in (2, 3):
                        nc.tensor.matmul(
                            pvs[j][:], vv_sb[:, k, j, :], visT[:, k, :],
                            start=(F == 0 and k == 0), stop=False,
                        )
                # hid(i)-dependent part of the vis field, then mix + AG per
                # j-pair; the vv j2/j3 tail (L tiles) runs between the two
                # halves as cover for AG_v_A
                for half in range(2):
                    js = (0, 1) if half == 0 else (2, 3)
                    for k in range(KT_H):
                        for j in js:
                            nc.tensor.matmul(
                                pvs[j][:], vht_sb[:, k, j, :], hidT[:, k, :],
                                start=False, stop=(k == KT_H - 1),
                            )
                    if half == 0:
                        for k in range(KT_V - L, KT_V):
                            for j in (2, 3):
                                nc.tensor.matmul(
                                    pvs[j][:], vv_sb[:, k, j, :], visT[:, k, :],
                                    start=False, stop=False,
                                )
                    for j in js:
                        pv = pvs[j]
                        prob = actpool.tile(
                            [128, BATCH], F32, name=f"prv{i}_{j}", tag="pr"
                        )
                        nc.scalar.activation(
                            prob[:], pv[:], SIG,
                            bias=vbt_sb[:, j, i : i + 1], scale=inv_t,
                        )
                        tmp = actpool.tile(
                            [128, BATCH], F32, name=f"tmv{i}_{j}", tag="tm"
                        )
                        nc.vector.tensor_sub(tmp[:], prob[:], vmyT[:, j, :])
                        nc.vector.scalar_tensor_tensor(
                            vmyT[:, j, :], tmp[:], 0.1, vmyT[:, j, :], MULT, ADD
                        )
                    if last or no_comm:
                        continue
                    ag_out = dram.tile(
                        [H_SIZE, BATCH], F32, addr_space=shared_as,
                        name=f"agov{i}_{half}", tag="agov",
                    )
                    all_gather(
                        vmyT[:, 2 * half : 2 * half + 2, :], ag_out,
                        scr_v[half],
                        nc.sync if half == 0 else nc.scalar,
                    )
                    qs = 0
                    for w in RESTAGE_V:
                        nc.sync.dma_start(
                            visW[:, KV2 * half + qs : KV2 * half + qs + w, :],
                            ag_out[:].rearrange("(k p) n -> p k n", p=128)[
                                :, qs : qs + w, :
                            ],
                        )
                        qs += w

            nc.sync.dma_start(
                out_vis[:].rearrange("(k p) n -> p k n", p=128), vmyT[:]
            )

    nc.compile()
    return nc


# vis k-tile permutation: SBUF order k' = AllGather output order.
# k' in [0,16): half A = each core's feature tiles {0,1};  orig k = 4c+t
# k' in [16,32): half B = tiles {2,3};                      orig k = 4c+2+t
_PERM_V = [4 * (k % 16 // 2) + (2 * (k // 16)) + (k % 2) for k in range(32)]
# hid k-tile permutation: per-j AllGather j=0 gathers each core's tile 0
# (orig 2c) into k' = c, j=1 gathers tile 1 (orig 2c+1) into k' = 8+c
_PERM_H = [2 * k for k in range(8)] + [2 * k + 1 for k in range(8)]


def _permute_vis_rows(a):
    """Reorder 128-row blocks of a (4096, ...) array into gather order."""
    blocks = a.reshape(32, 128, *a.shape[1:])
    return np.ascontiguousarray(blocks[_PERM_V].reshape(a.shape))


def _permute_hid_rows(a):
    """Reorder 128-row blocks of a (2048, ...) array into gather order."""
    blocks = a.reshape(16, 128, *a.shape[1:])
    return np.ascontiguousarray(blocks[_PERM_H].reshape(a.shape))


def _prep_inputs(x, vis_bias, hid_bias, vis_hid, vis_vis_raw, hid_hid_raw,
                 temps):
    f32 = np.float32
    n_steps = len(temps)
    vv = np.triu(np.asarray(vis_vis_raw, dtype=f32), 1)
    vv = vv + vv.T
    hh = np.triu(np.asarray(hid_hid_raw, dtype=f32), 1)
    hh = hh + hh.T
    vis_hid = np.ascontiguousarray(np.asarray(vis_hid, dtype=f32))
    vht = np.ascontiguousarray(vis_hid.T)  # (H, V)
    x = np.asarray(x, dtype=f32)
    xT = np.ascontiguousarray(x.T)
    hid0 = np.full((H_SIZE, BATCH), 0.5, dtype=f32)
    hb = np.asarray(hid_bias, dtype=f32)
    vb = np.asarray(vis_bias, dtype=f32)
    inv_t = (1.0 / temps).astype(f32)  # [n_steps]

    in_maps = []
    for c in range(N_CORES):
        hsl = slice(c * HS, (c + 1) * HS)
        vsl = slice(c * VS, (c + 1) * VS)
        # bias_over_temp tables [128, NT, n_steps] -> flat [128, NT*n_steps]
        hbt = (
            hb[hsl].reshape(NTH, 128).T[:, :, None] * inv_t[None, None, :]
        ).astype(f32)
        vbt = (
            vb[vsl].reshape(NTV, 128).T[:, :, None] * inv_t[None, None, :]
        ).astype(f32)
        in_maps.append(
            {
                "xT": _permute_vis_rows(xT),
                "xT_my": np.ascontiguousarray(xT[vsl]),
                "hid0T": hid0,
                "hh_w": np.ascontiguousarray(hh[:, hsl]),
                "vh_w": _permute_vis_rows(np.ascontiguousarray(vis_hid[:, hsl])),
                "vv_w": _permute_vis_rows(np.ascontiguousarray(vv[:, vsl])),
                "vht_w": np.ascontiguousarray(vht[:, vsl]),
                "hbt_in": np.ascontiguousarray(hbt.reshape(128, NTH * n_steps)),
                "vbt_in": np.ascontiguousarray(vbt.reshape(128, NTV * n_steps)),
            }
        )
    return in_maps


def kernel(
    x,
    vis_bias,
    hid_bias,
    vis_hid,
    vis_vis_raw,
    hid_hid_raw,
    max_steps,
):
    from concourse import bass_utils

    n_steps = int(max_steps)
    steps_f = np.float32(n_steps)
    temps = (
        np.float32(0.01)
        * (
            np.float32(1.0)
            + np.float32(4.0)
            * np.exp(
                np.float32(-5.0)
                * np.arange(n_steps, dtype=np.float32)
                / steps_f
            )
        )
    ).astype(np.float32)

    if n_steps not in _BUILT:
        _BUILT[n_steps] = _build(n_steps, temps)
    nc = _BUILT[n_steps]

    in_maps = _prep_inputs(
        x, vis_bias, hid_bias, vis_hid, vis_vis_raw, hid_hid_raw, temps
    )
    res = bass_utils.run_bass_kernel_spmd(
        nc, in_maps, core_ids=list(range(N_CORES))
    )

    out = np.empty((BATCH, V_SIZE), dtype=np.float32)
    for c in range(N_CORES):
        out[:, c * VS : (c + 1) * VS] = res.results[c]["vis_shT"].T
    kernel._last_result = res
    return out

